# revision 1
# baseline (speedup 1.0000x reference)
"""Trainium2 Bass kernel for nn_DSGCNLayer (gnn_message_passing) — v2.

Math (same folding as v1): the reference's einsum contracts m only within
A_eff, so the GCN+depthwise-k+pointwise stack collapses into one per-node
128->256 GEMM with 3 temporal taps PSUM-accumulated against t-shifted views
of x^T (tap-scaled weights W2[n]).

v2 changes vs v1 (all aimed at per-dispatch overhead + engine balance):
  * All parameters (w2, wres, ident, gn/ln affine) are embedded in the NEFF
    via inline_tensor — the only runtime input is xt (6.2 MB/core vs 16 MB),
    cutting the per-call input-staging overhead of the axon dispatch path.
  * Output is bf16 in a [N, p, j, o] layout (2 KB contiguous per partition
    per node); host reassembles/upcasts. Halves output DMA.
  * GN/LN statistics come from DVE bn_stats (one pass, mean+M2) instead of
    copy-accum + ACT/DVE square passes.
  * PSUM->SBUF h copy moved to ACT (single Copy op), GN apply moved to the
    otherwise-idle Pool/GPSIMD engine, y copy is a single DVE op.
Engine budget per node: PE 18 matmuls, ACT 1 copy + 4 gelu, Pool 2 apply,
DVE 4 bn_stats + 1 copy (+ per-group finalize)."""

import hashlib

import numpy as np
import ml_dtypes

import concourse.bass as bass
import concourse.mybir as mybir
import concourse.tile as tile
from concourse.bass_utils import run_bass_kernel_spmd

BF16 = mybir.dt.bfloat16
F32 = mybir.dt.float32
I32 = mybir.dt.int32
AF = mybir.ActivationFunctionType
ALU = mybir.AluOpType

B, T, N, C_IN, C_OUT, K, KT, GN_GROUPS = 8, 512, 47, 128, 256, 3, 3, 8
EPS = 1e-5
NT = T // 128          # 4 t-tiles per node
GROUP = 4              # nodes per stats batch
N_CORES = 8
MAGIC = 0x5F3759DF


def _fix_multiwaits(nc, max_waits: int = 1) -> int:
    """The walrus build in this container rejects instructions carrying more
    than one sync-wait command.  Move excess sem-waits onto same-engine
    InstNoOp carriers inserted immediately before the instruction."""
    n_fixed = 0
    for fn in nc.m.functions:
        for bb in fn.blocks:
            insts = bb.instructions
            out = []
            changed = False
            for inst in insts:
                si = inst.sync_info
                if si is not None and len(si.on_wait) > max_waits:
                    waits = list(si.on_wait)
                    extra, keep = waits[:-max_waits], waits[-max_waits:]
                    for j in range(0, len(extra), max_waits):
                        nop = mybir.InstNoOp(
                            name=nc.get_next_instruction_name(), ins=[], outs=[]
                        )
                        nop.engine = inst.engine
                        nop.sync_info = mybir.SyncInfo(
                            on_wait=extra[j : j + max_waits], on_update=[]
                        )
                        out.append(nop)
                    inst.sync_info = mybir.SyncInfo(
                        on_wait=keep, on_update=list(si.on_update)
                    )
                    changed = True
                    n_fixed += 1
                out.append(inst)
            if changed:
                bb.instructions = out
    return n_fixed


def _newton_rsqrt(nc, pool, veps, magic, width):
    """rsqrt(veps) on DVE: Quake initial guess + 2 Newton iterations."""
    sh = pool.tile([128, width], I32, tag=f"nw_sh{width}")
    nc.vector.tensor_scalar(
        out=sh, in0=veps.bitcast(I32), scalar1=1, scalar2=None,
        op0=ALU.arith_shift_right,
    )
    ri = pool.tile([128, width], I32, tag=f"nw_ri{width}")
    nc.vector.scalar_tensor_tensor(
        out=ri, in0=magic[:, :width], scalar=0, in1=sh,
        op0=ALU.bypass, op1=ALU.subtract,
    )
    cur = ri.bitcast(F32)
    t1 = pool.tile([128, width], F32, tag=f"nw_t1{width}")
    t2 = pool.tile([128, width], F32, tag=f"nw_t2{width}")
    for it in range(1):
        dst = pool.tile([128, width], F32, tag=f"nw_r{it}_{width}")
        nc.vector.tensor_tensor(out=t1, in0=veps, in1=cur, op=ALU.mult)
        nc.vector.tensor_tensor(out=t2, in0=t1, in1=cur, op=ALU.mult)
        nc.vector.tensor_scalar(
            out=t1, in0=t2, scalar1=-0.5, scalar2=1.5, op0=ALU.mult, op1=ALU.add
        )
        nc.vector.tensor_tensor(out=dst, in0=cur, in1=t1, op=ALU.mult)
        cur = dst
    return cur


def _build_nc(consts: dict, gn_beta_nonzero: bool, ln_nontrivial: bool,
              repeat: int = 1):
    nc = bass.Bass()

    xt_in = nc.declare_dram_parameter("xt", [N, 128, 514], BF16, isOutput=False)
    out_d = nc.declare_dram_parameter("out", [N, 128, NT, C_OUT], BF16, isOutput=True)

    w2_c = nc.inline_tensor(consts["w2"], name="w2c")          # [N,128,KT,C_OUT] bf16
    wres_c = nc.inline_tensor(consts["wres"], name="wresc")    # [128,C_OUT] bf16
    id_c = nc.inline_tensor(consts["ident"], name="identc")    # [128,128] f32
    gng_c = nc.inline_tensor(consts["gng"], name="gngc")       # [128,GROUP*2] f32
    gnb_c = nc.inline_tensor(consts["gnb"], name="gnbc") if gn_beta_nonzero else None
    lng_c = nc.inline_tensor(consts["lng"], name="lngc") if ln_nontrivial else None
    lnb_c = nc.inline_tensor(consts["lnb"], name="lnbc") if ln_nontrivial else None

    n_groups = (N + GROUP - 1) // GROUP

    with tile.TileContext(nc) as tc:
        with (
            tc.tile_pool(name="const", bufs=1) as cp,
            tc.tile_pool(name="xt", bufs=2 * GROUP + 2) as xt_pool,
            tc.tile_pool(name="w2", bufs=4) as w2_pool,
            tc.tile_pool(name="hsb", bufs=2 * GROUP + 2) as hsb_pool,
            tc.tile_pool(name="hgn", bufs=3) as hgn_pool,
            tc.tile_pool(name="ysb", bufs=2 * GROUP + 2) as y_pool,
            tc.tile_pool(name="osb", bufs=3) as out_pool,
            tc.tile_pool(name="scr", bufs=2) as scr_pool,
            tc.tile_pool(name="stats", bufs=2) as st_pool,
            tc.tile_pool(name="fin", bufs=2) as fin_pool,
            tc.tile_pool(name="ph", bufs=2, space="PSUM") as ph_pool,
            tc.tile_pool(name="py", bufs=2, space="PSUM") as py_pool,
        ):
            wres_t = cp.tile([128, C_OUT], BF16)
            nc.sync.dma_start(out=wres_t, in_=wres_c[:, :])
            ident_t = cp.tile([128, 128], F32)
            nc.sync.dma_start(out=ident_t, in_=id_c[:, :])
            gng_t = cp.tile([128, GROUP, 2], F32)
            nc.sync.dma_start(out=gng_t, in_=gng_c[:, 0 : GROUP * 2].rearrange("p (n h) -> p n h", h=2))
            gnb_t = None
            if gn_beta_nonzero:
                gnb_t = cp.tile([128, GROUP, 2], F32)
                nc.sync.dma_start(out=gnb_t, in_=gnb_c[:, 0 : GROUP * 2].rearrange("p (n h) -> p n h", h=2))
            lng_t = lnb_t = None
            if ln_nontrivial:
                lng_t = cp.tile([128, C_OUT], F32)
                nc.sync.dma_start(out=lng_t, in_=lng_c[:, :])
                lnb_t = cp.tile([128, C_OUT], F32)
                nc.sync.dma_start(out=lnb_t, in_=lnb_c[:, :])
            ones32 = cp.tile([128, 32], F32)
            nc.vector.memset(ones32, 1.0)
            magic = cp.tile([128, 32], I32)
            nc.vector.memset(magic, MAGIC)

            def emit_phase1(g):
                nodes = list(range(g * GROUP, min(N, (g + 1) * GROUP)))
                nl = len(nodes)
                bnh = st_pool.tile([128, GROUP, 2, 6], F32, tag="bnh")
                lnbuf = st_pool.tile([128, GROUP, NT, 6], F32, tag="lnbuf")
                if nl < GROUP:
                    nc.vector.memset(bnh, 0.0)
                    nc.vector.memset(lnbuf, 0.0)

                xts, hsbs = [], []
                # GEMM, h copy (ACT), bn_stats (DVE)
                for i, n in enumerate(nodes):
                    xt_t = xt_pool.tile([128, 514], BF16, tag="xt")
                    nc.sync.dma_start(out=xt_t, in_=xt_in[n])
                    w2_t = w2_pool.tile([128, KT, C_OUT], BF16, tag="w2")
                    nc.sync.dma_start(out=w2_t, in_=w2_c[n])
                    xts.append(xt_t)

                    ph = ph_pool.tile([128, 2, T], F32, tag="ph")
                    for h in range(2):
                        for tap in range(KT):
                            nc.tensor.matmul(
                                ph[:, h, :],
                                lhsT=w2_t[:, tap, h * 128 : (h + 1) * 128],
                                rhs=xt_t[:, tap : tap + T],
                                start=(tap == 0),
                                stop=(tap == KT - 1),
                            )
                    h_sb = hsb_pool.tile([128, 2, T], BF16, tag="hsb")
                    hsbs.append(h_sb)
                    nc.scalar.activation(out=h_sb, in_=ph, func=AF.Copy)
                    for h in range(2):
                        nc.vector.bn_stats(out=bnh[:, i, h, :], in_=h_sb[:, h, :])
                return dict(nodes=nodes, bnh=bnh, lnbuf=lnbuf, xts=xts, hsbs=hsbs)

            def emit_rest(state):
                nodes, bnh, lnbuf = state["nodes"], state["bnh"], state["lnbuf"]
                xts, hsbs = state["xts"], state["hsbs"]

                # ---- GN finalize (from bn_stats even/odd moments) -----------
                # E[h]  = (me+mo)/2 ; E[h^2] = (cve+cvo)/512 + (me^2+mo^2)/2
                # St columns hold 2*E[h] and 2*E[h^2]; after the 32-partition
                # group-sum, scaling by 1/64 yields group E[h], E[h^2].
                me = bnh[:, :, :, 1]
                mo = bnh[:, :, :, 4]
                cve = bnh[:, :, :, 2]
                cvo = bnh[:, :, :, 5]
                St32 = fin_pool.tile([128, 32], F32, tag="St32")
                nc.vector.memset(St32[:, GROUP * 4 :], 0.0)
                St = St32[:, 0 : GROUP * 4].rearrange("p (n h m) -> p n h m", h=2, m=2)
                nc.vector.tensor_tensor(out=St[:, :, :, 0], in0=me, in1=mo, op=ALU.add)
                b_t = fin_pool.tile([128, GROUP, 2], F32, tag="bt")
                nc.vector.tensor_tensor(out=b_t, in0=cve, in1=cvo, op=ALU.add)
                c_t = fin_pool.tile([128, GROUP, 2], F32, tag="ct")
                nc.vector.tensor_tensor(out=c_t, in0=me, in1=me, op=ALU.mult)
                d_t = fin_pool.tile([128, GROUP, 2], F32, tag="dt")
                nc.vector.tensor_tensor(out=d_t, in0=mo, in1=mo, op=ALU.mult)
                e_t = fin_pool.tile([128, GROUP, 2], F32, tag="et")
                nc.vector.tensor_tensor(out=e_t, in0=c_t, in1=d_t, op=ALU.add)
                nc.vector.scalar_tensor_tensor(
                    out=St[:, :, :, 1], in0=b_t, scalar=1.0 / 256.0, in1=e_t,
                    op0=ALU.mult, op1=ALU.add,
                )
                Tr = fin_pool.tile([128, 32], F32, tag="Tr")
                nc.vector.transpose(out=Tr, in_=St32)
                R = fin_pool.tile([128, 1], F32, tag="R")
                nc.vector.tensor_reduce(out=R, in_=Tr, axis=mybir.AxisListType.X, op=ALU.add)
                Rep = fin_pool.tile([128, 32], F32, tag="Rep")
                nc.vector.tensor_scalar(
                    out=Rep, in0=ones32, scalar1=R[:, 0:1], scalar2=None, op0=ALU.mult
                )
                M0 = fin_pool.tile([128, 32], F32, tag="M0")
                nc.vector.transpose(out=M0, in_=Rep)
                Mn = fin_pool.tile([128, 32], F32, tag="Mn")
                nc.vector.tensor_scalar(
                    out=Mn, in0=M0, scalar1=1.0 / 64.0, scalar2=None, op0=ALU.mult
                )
                MnV = Mn[:, 0 : GROUP * 4].rearrange("p (n s) -> p n s", s=4)
                mu = MnV[:, :, 0::2]      # [128, 8, 2]
                E2 = MnV[:, :, 1::2]
                musq = fin_pool.tile([128, GROUP, 2], F32, tag="musq")
                nc.vector.tensor_tensor(out=musq, in0=mu, in1=mu, op=ALU.mult)
                veps = fin_pool.tile([128, GROUP, 2], F32, tag="veps")
                nc.vector.scalar_tensor_tensor(
                    out=veps, in0=E2, scalar=EPS, in1=musq,
                    op0=ALU.add, op1=ALU.subtract,
                )
                rstd = _newton_rsqrt(nc, fin_pool, veps.rearrange("p n s -> p (n s)"), magic, GROUP * 2)
                rstdV = rstd.rearrange("p (n s) -> p n s", s=2)
                gnsc = fin_pool.tile([128, GROUP * 2], F32, tag="gnsc")
                gnscV = gnsc.rearrange("p (n h) -> p n h", h=2)
                nc.vector.tensor_tensor(out=gnscV, in0=rstdV, in1=gng_t, op=ALU.mult)
                gnbi = fin_pool.tile([128, GROUP * 2], F32, tag="gnbi")
                gnbiV = gnbi.rearrange("p (n h) -> p n h", h=2)
                nc.vector.scalar_tensor_tensor(
                    out=gnbiV, in0=mu, scalar=-1.0, in1=gnscV, op0=ALU.mult, op1=ALU.mult
                )
                if gn_beta_nonzero:
                    nc.vector.tensor_tensor(out=gnbiV, in0=gnbiV, in1=gnb_t, op=ALU.add)

                # ---- phase 2: GN apply (Pool), y assembly (PE), stats -------
                ysbs = []
                for i, n in enumerate(nodes):
                    xt_t, h_sb = xts[i], hsbs[i]
                    h_gn = hgn_pool.tile([128, 2, T], F32, tag="hgn")
                    for h in range(2):
                        nc.gpsimd.tensor_scalar(
                            out=h_gn[:, h, :], in0=h_sb[:, h, :],
                            scalar1=gnsc[:, 2 * i + h : 2 * i + h + 1],
                            scalar2=gnbi[:, 2 * i + h : 2 * i + h + 1],
                            op0=ALU.mult, op1=ALU.add,
                        )
                    py = py_pool.tile([128, NT, C_OUT], F32, tag="py")
                    for j in range(NT):
                        nc.tensor.matmul(
                            py[:, j, :],
                            lhsT=xt_t[:, 1 + 128 * j : 1 + 128 * (j + 1)],
                            rhs=wres_t,
                            start=(j % 2 == 0), stop=False, skip_group_check=True,
                        )
                    for j in range(NT):
                        for h in range(2):
                            nc.tensor.matmul(
                                py[:, j, h * 128 : (h + 1) * 128],
                                lhsT=h_gn[:, h, 128 * j : 128 * (j + 1)],
                                rhs=ident_t,
                                is_transpose=True,
                                start=False, stop=(h == 1), skip_group_check=True,
                            )
                    y_sb = y_pool.tile([128, NT, C_OUT], BF16, tag="ysb")
                    ysbs.append(y_sb)
                    nc.scalar.activation(out=y_sb, in_=py, func=AF.Copy)
                    for j in range(NT):
                        nc.vector.bn_stats(out=lnbuf[:, i, j, :], in_=y_sb[:, j, :])

                # ---- LN finalize --------------------------------------------
                # per (node, j): mu = (me+mo)/2 ; E2 = (cve+cvo)/256+(me^2+mo^2)/2
                lme = lnbuf[:, :, :, 1]
                lmo = lnbuf[:, :, :, 4]
                lcve = lnbuf[:, :, :, 2]
                lcvo = lnbuf[:, :, :, 5]
                a_l = fin_pool.tile([128, GROUP, NT], F32, tag="al")
                nc.vector.tensor_tensor(out=a_l, in0=lme, in1=lmo, op=ALU.add)
                mu_l = fin_pool.tile([128, GROUP, NT], F32, tag="mul")
                nc.vector.tensor_scalar(
                    out=mu_l, in0=a_l, scalar1=0.5, scalar2=None, op0=ALU.mult
                )
                b_l = fin_pool.tile([128, GROUP, NT], F32, tag="bl")
                nc.vector.tensor_tensor(out=b_l, in0=lcve, in1=lcvo, op=ALU.add)
                c_l = fin_pool.tile([128, GROUP, NT], F32, tag="cl")
                nc.vector.tensor_tensor(out=c_l, in0=lme, in1=lme, op=ALU.mult)
                d_l = fin_pool.tile([128, GROUP, NT], F32, tag="dl")
                nc.vector.tensor_tensor(out=d_l, in0=lmo, in1=lmo, op=ALU.mult)
                e_l = fin_pool.tile([128, GROUP, NT], F32, tag="el")
                nc.vector.tensor_tensor(out=e_l, in0=c_l, in1=d_l, op=ALU.add)
                g_l = fin_pool.tile([128, GROUP, NT], F32, tag="gl")
                nc.vector.scalar_tensor_tensor(
                    out=g_l, in0=b_l, scalar=1.0 / 128.0, in1=e_l,
                    op0=ALU.mult, op1=ALU.add,
                )   # = 2*E[y^2]
                musq_l = fin_pool.tile([128, GROUP, NT], F32, tag="musql")
                nc.vector.tensor_tensor(out=musq_l, in0=mu_l, in1=mu_l, op=ALU.mult)
                musq2_l = fin_pool.tile([128, GROUP, NT], F32, tag="musq2l")
                nc.vector.tensor_scalar(
                    out=musq2_l, in0=musq_l, scalar1=1.0, scalar2=EPS,
                    op0=ALU.mult, op1=ALU.subtract,
                )   # = mu^2 - EPS
                veps_l = fin_pool.tile([128, GROUP, NT], F32, tag="vepsl")
                nc.vector.scalar_tensor_tensor(
                    out=veps_l, in0=g_l, scalar=0.5, in1=musq2_l,
                    op0=ALU.mult, op1=ALU.subtract,
                )   # = E[y^2] - mu^2 + EPS
                rsig = _newton_rsqrt(
                    nc, fin_pool, veps_l.rearrange("p n j -> p (n j)"), magic, GROUP * NT
                )
                rsigV = rsig.rearrange("p (n j) -> p n j", j=NT)
                nb = fin_pool.tile([128, GROUP * NT], F32, tag="nb")
                nbV = nb.rearrange("p (n j) -> p n j", j=NT)
                nc.vector.scalar_tensor_tensor(
                    out=nbV, in0=mu_l, scalar=-1.0, in1=rsigV, op0=ALU.mult, op1=ALU.mult
                )

                # ---- phase 3: LN apply + gelu + store -----------------------
                for i, n in enumerate(nodes):
                    y_sb = ysbs[i]
                    o_sb = out_pool.tile([128, NT, C_OUT], BF16, tag="osb")
                    if not ln_nontrivial:
                        for j in range(NT):
                            nc.scalar.activation(
                                out=o_sb[:, j, :], in_=y_sb[:, j, :], func=AF.Gelu,
                                scale=rsig[:, NT * i + j : NT * i + j + 1],
                                bias=nb[:, NT * i + j : NT * i + j + 1],
                            )
                    else:
                        u = scr_pool.tile([128, NT, C_OUT], F32, tag="u")
                        for j in range(NT):
                            nc.vector.tensor_scalar(
                                out=u[:, j, :], in0=y_sb[:, j, :],
                                scalar1=rsig[:, NT * i + j : NT * i + j + 1],
                                scalar2=nb[:, NT * i + j : NT * i + j + 1],
                                op0=ALU.mult, op1=ALU.add,
                            )
                            nc.vector.tensor_tensor(
                                out=u[:, j, :], in0=u[:, j, :], in1=lng_t, op=ALU.mult
                            )
                            nc.vector.tensor_tensor(
                                out=u[:, j, :], in0=u[:, j, :], in1=lnb_t, op=ALU.add
                            )
                            nc.scalar.activation(
                                out=o_sb[:, j, :], in_=u[:, j, :], func=AF.Gelu
                            )
                    nc.sync.dma_start(out=out_d[n], in_=o_sb)

            # Software pipeline: issue group g+1 phase-1 before group g's
            # finalize/phase-2/phase-3 so engines stay busy across the
            # group-level stats barriers.
            pending = None
            for rep in range(repeat):
                for g in range(n_groups):
                    cur = emit_phase1(g)
                    if pending is not None:
                        emit_rest(pending)
                    pending = cur
            if pending is not None:
                emit_rest(pending)

    _fix_multiwaits(nc)
    return nc


_CACHE: dict = {}


def prepare(x, A, adj_residual, dw_weights, W_pw, W_conv, gn_gamma, gn_beta,
            ln_gamma, ln_beta, W_res):
    """Host-side parameter folding + input staging. Returns (nc, in_maps)."""
    x = np.asarray(x, np.float32)
    A = np.asarray(A, np.float32)
    adj_residual = np.asarray(adj_residual, np.float32)
    dw_weights = np.asarray(dw_weights, np.float32)
    W_pw = np.asarray(W_pw, np.float32)
    W_conv = np.asarray(W_conv, np.float32)
    gn_gamma = np.asarray(gn_gamma, np.float32)
    gn_beta = np.asarray(gn_beta, np.float32)
    ln_gamma = np.asarray(ln_gamma, np.float32)
    ln_beta = np.asarray(ln_beta, np.float32)
    W_res = np.asarray(W_res, np.float32)

    A_eff = A + np.tanh(adj_residual) * 0.3
    A_eff = A_eff / np.clip(np.abs(A_eff).sum(-1, keepdims=True), 1.0, None)
    rowsum = A_eff.sum(-1)                                   # [K, N]
    W_pw_r = W_pw.reshape(C_OUT, K, C_IN)                    # [o, k, c]
    W_effT = np.einsum("kn,kc,okc->nco", rowsum, dw_weights, W_pw_r)
    WC = W_conv[:, 0, :]                                     # [o, tap]
    W2 = W_effT[:, :, None, :] * WC.T[None, None, :, :]      # [n, c, tap, o]
    w2_host = np.ascontiguousarray(W2).astype(ml_dtypes.bfloat16)

    xt = np.zeros((B, N, 128, 514), np.float32)
    xt[:, :, :, 1:513] = np.transpose(x, (0, 2, 3, 1))       # [b, n, c, t]
    xt_host = xt.astype(ml_dtypes.bfloat16)

    wres_host = np.ascontiguousarray(W_res.T).astype(ml_dtypes.bfloat16)
    ident = np.eye(128, dtype=np.float32)

    p = np.arange(128)
    gng = np.zeros((128, GROUP, 2), np.float32)
    gnb = np.zeros((128, GROUP, 2), np.float32)
    for h in range(2):
        gng[:, :, h] = gn_gamma[h * 128 + p][:, None]
        gnb[:, :, h] = gn_beta[h * 128 + p][:, None]
    gng = gng.reshape(128, GROUP * 2)
    gnb = gnb.reshape(128, GROUP * 2)

    ln_nontrivial = not (np.all(ln_gamma == 1.0) and np.all(ln_beta == 0.0))
    gn_beta_nonzero = bool(np.any(gn_beta != 0.0))
    lng = np.broadcast_to(ln_gamma[None, :], (128, C_OUT)).astype(np.float32).copy()
    lnb = np.broadcast_to(ln_beta[None, :], (128, C_OUT)).astype(np.float32).copy()

    consts = {
        "w2": w2_host, "wres": wres_host, "ident": ident,
        "gng": gng, "gnb": gnb, "lng": lng, "lnb": lnb,
    }
    hsh = hashlib.sha1()
    for k in sorted(consts):
        hsh.update(np.ascontiguousarray(consts[k]).tobytes())
    key = (gn_beta_nonzero, ln_nontrivial, hsh.hexdigest())
    if key not in _CACHE:
        _CACHE[key] = _build_nc(consts, gn_beta_nonzero, ln_nontrivial)
    nc = _CACHE[key]

    in_maps = [{"xt": xt_host[b]} for b in range(B)]
    return nc, in_maps


def _warm_devices():
    """A previously crashed process can leave the remote NeuronCores in an
    unrecoverable state; run a sacrificial tiny op (with retries) so the real
    launch lands on healthy devices."""
    import time as _time
    import jax
    import jax.numpy as jnp
    for attempt in range(4):
        try:
            for d in jax.devices()[:N_CORES]:
                y = jax.device_put(jnp.ones((2,)), d) + 1.0
                np.asarray(y)
            return
        except Exception:
            _time.sleep(4.0)


def kernel(x, A, adj_residual, dw_weights, W_pw, W_conv, gn_gamma, gn_beta,
           ln_gamma, ln_beta, W_res):
    nc, in_maps = prepare(x, A, adj_residual, dw_weights, W_pw, W_conv,
                          gn_gamma, gn_beta, ln_gamma, ln_beta, W_res)
    _warm_devices()
    import time as _time
    last_err = None
    for attempt in range(2):
        try:
            res = run_bass_kernel_spmd(nc, in_maps, core_ids=list(range(N_CORES)))
            break
        except Exception as e:
            last_err = e
            _time.sleep(4.0)
            _warm_devices()
    else:
        raise last_err
    # out per core: [N, p, j, o] bf16; t = j*128 + p
    outs = []
    for b in range(B):
        arr = np.asarray(res.results[b]["out"])      # [N, 128, NT, C_OUT]
        arr = arr.transpose(2, 1, 0, 3).reshape(T, N, C_OUT)
        outs.append(arr)
    return np.stack(outs, axis=0).astype(np.float32)



# revision 34
# speedup vs baseline: 11.3434x; 11.3434x over previous
"""Trainium2 Bass kernel for nn_DSGCNLayer (gnn_message_passing) — v3.

Math (same folding as v1/v2): the reference's einsum contracts m only within
A_eff, so the GCN+depthwise-k+pointwise stack collapses into one per-node
128->256 GEMM with 3 temporal taps PSUM-accumulated against t-shifted views
of x^T (tap-scaled weights W2[n]).

v3 changes vs v2 (engine rebalance guided by the instruction cost model):
  * GROUP=8 (was 4): the GN stats partition-block transpose trick exactly
    fills its 32 columns, and per-group finalize overhead halves.
  * GN apply moved Pool->DVE tensor_scalar (bf16 SBUF<->SBUF runs in 4x DVE
    mode: ~0.26ns/el vs Pool's ~1.6ns/el), applied in-place on h_sb.
  * h_sb stays bf16 into the PE transposes (1 cycle/row vs 2 for f32) with a
    bf16 identity.
  * y copy (PSUM->SBUF) moved ACT->Pool; ACT keeps only the h copy + gelu.
  * GN/LN finalize elementwise chains moved DVE->Pool (DVE keeps the two
    32x32 block transposes); Newton rsqrt runs on Pool.
  * xt/w2/out DMAs batched 2 nodes per descriptor (fewer SP/HWDGE slots).
Engine budget per node (el/lane): ACT h-copy 1024 + gelu 1024; DVE bn_stats
2048 + GN apply 1024 (4x); Pool y-copy 1024 + finalize; PE 18 matmuls."""

import hashlib

import numpy as np
import ml_dtypes

import concourse.bass as bass
import concourse.mybir as mybir
import concourse.tile as tile
from concourse.bass_utils import run_bass_kernel_spmd

BF16 = mybir.dt.bfloat16
F32 = mybir.dt.float32
I32 = mybir.dt.int32
AF = mybir.ActivationFunctionType
ALU = mybir.AluOpType

B, T, N, C_IN, C_OUT, K, KT, GN_GROUPS = 8, 512, 47, 128, 256, 3, 3, 8
EPS = 1e-5
NT = T // 128          # 4 t-tiles per node
GROUP = 8              # nodes per stats batch
N_CORES = 8
MAGIC = 0x5F3759DF


def _fix_multiwaits(nc, max_waits: int = 1) -> int:
    """The walrus build in this container rejects instructions carrying more
    than one sync-wait command.  Move excess sem-waits onto same-engine
    InstNoOp carriers inserted immediately before the instruction."""
    n_fixed = 0
    for fn in nc.m.functions:
        for bb in fn.blocks:
            insts = bb.instructions
            out = []
            changed = False
            for inst in insts:
                si = inst.sync_info
                if si is not None and len(si.on_wait) > max_waits:
                    waits = list(si.on_wait)
                    extra, keep = waits[:-max_waits], waits[-max_waits:]
                    for j in range(0, len(extra), max_waits):
                        nop = mybir.InstNoOp(
                            name=nc.get_next_instruction_name(), ins=[], outs=[]
                        )
                        nop.engine = inst.engine
                        nop.sync_info = mybir.SyncInfo(
                            on_wait=extra[j : j + max_waits], on_update=[]
                        )
                        out.append(nop)
                    inst.sync_info = mybir.SyncInfo(
                        on_wait=keep, on_update=list(si.on_update)
                    )
                    changed = True
                    n_fixed += 1
                out.append(inst)
            if changed:
                bb.instructions = out
    return n_fixed


def _newton_rsqrt(nc, eng, pool, veps, magic, width):
    """rsqrt(veps): Quake initial guess + 1 Newton iteration, on engine
    namespace `eng` (nc.vector or nc.gpsimd)."""
    sh = pool.tile([128, width], I32, tag=f"nw_sh{width}")
    eng.tensor_scalar(
        out=sh, in0=veps.bitcast(I32), scalar1=1, scalar2=None,
        op0=ALU.arith_shift_right,
    )
    ri = pool.tile([128, width], I32, tag=f"nw_ri{width}")
    eng.scalar_tensor_tensor(
        out=ri, in0=magic[:, :width], scalar=0, in1=sh,
        op0=ALU.bypass, op1=ALU.subtract,
    )
    cur = ri.bitcast(F32)
    t1 = pool.tile([128, width], F32, tag=f"nw_t1{width}")
    t2 = pool.tile([128, width], F32, tag=f"nw_t2{width}")
    for it in range(1):
        dst = pool.tile([128, width], F32, tag=f"nw_r{it}_{width}")
        eng.tensor_tensor(out=t1, in0=veps, in1=cur, op=ALU.mult)
        eng.tensor_tensor(out=t2, in0=t1, in1=cur, op=ALU.mult)
        eng.tensor_scalar(
            out=t1, in0=t2, scalar1=-0.5, scalar2=1.5, op0=ALU.mult, op1=ALU.add
        )
        eng.tensor_tensor(out=dst, in0=cur, in1=t1, op=ALU.mult)
        cur = dst
    return cur


def _build_nc(consts: dict, gn_beta_nonzero: bool, ln_nontrivial: bool,
              repeat: int = 1):
    nc = bass.Bass()

    xt_in = nc.declare_dram_parameter("xt", [N, 128, 514], BF16, isOutput=False)
    out_d = nc.declare_dram_parameter("out", [N, 128, NT, C_OUT], BF16, isOutput=True)

    w2_c = nc.inline_tensor(consts["w2"], name="w2c")          # [N,128,KT,C_OUT] bf16
    wres_c = nc.inline_tensor(consts["wres"], name="wresc")    # [128,C_OUT] bf16
    id_c = nc.inline_tensor(consts["ident"], name="identc")    # [128,128] bf16
    gng_c = nc.inline_tensor(consts["gng"], name="gngc")       # [128,GROUP*2] f32
    gnb_c = nc.inline_tensor(consts["gnb"], name="gnbc") if gn_beta_nonzero else None
    lng_c = nc.inline_tensor(consts["lng"], name="lngc") if ln_nontrivial else None
    lnb_c = nc.inline_tensor(consts["lnb"], name="lnbc") if ln_nontrivial else None

    n_groups = (N + GROUP - 1) // GROUP

    with tile.TileContext(nc) as tc:
        with (
            tc.tile_pool(name="const", bufs=1) as cp,
            tc.tile_pool(name="xt", bufs=GROUP + 4) as xt_pool,        # pair tiles
            tc.tile_pool(name="w2", bufs=6) as w2_pool,                # pair tiles
            tc.tile_pool(name="hsb", bufs=2 * GROUP + 2) as hsb_pool,
            tc.tile_pool(name="ysb", bufs=2 * GROUP + 2) as y_pool,
            tc.tile_pool(name="osb", bufs=6) as out_pool,              # pair tiles
            tc.tile_pool(name="scr", bufs=8) as scr_pool,
            tc.tile_pool(name="stats", bufs=3) as st_pool,
            tc.tile_pool(name="fin", bufs=3) as fin_pool,
            tc.tile_pool(name="ph", bufs=2, space="PSUM") as ph_pool,
            tc.tile_pool(name="py", bufs=2, space="PSUM") as py_pool,
        ):
            wres_t = cp.tile([128, C_OUT], BF16)
            nc.scalar.dma_start(out=wres_t, in_=wres_c[:, :])
            ident_t = cp.tile([128, 128], BF16)
            nc.scalar.dma_start(out=ident_t, in_=id_c[:, :])
            gng_t = cp.tile([128, GROUP, 2], F32)
            nc.scalar.dma_start(out=gng_t, in_=gng_c[:, 0 : GROUP * 2].rearrange("p (n h) -> p n h", h=2))
            gnb_t = None
            if gn_beta_nonzero:
                gnb_t = cp.tile([128, GROUP, 2], F32)
                nc.scalar.dma_start(out=gnb_t, in_=gnb_c[:, 0 : GROUP * 2].rearrange("p (n h) -> p n h", h=2))
            lng_t = lnb_t = None
            if ln_nontrivial:
                lng_t = cp.tile([128, C_OUT], F32)
                nc.scalar.dma_start(out=lng_t, in_=lng_c[:, :])
                lnb_t = cp.tile([128, C_OUT], F32)
                nc.scalar.dma_start(out=lnb_t, in_=lnb_c[:, :])
            ones32 = cp.tile([128, 32], F32)
            nc.vector.memset(ones32, 1.0)
            magic = cp.tile([128, 32], I32)
            nc.vector.memset(magic, MAGIC)

            def emit_phase1(g):
                nodes = list(range(g * GROUP, min(N, (g + 1) * GROUP)))
                nl = len(nodes)
                bnh = st_pool.tile([128, GROUP, 2, 6], F32, tag="bnh")
                lnbuf = st_pool.tile([128, GROUP, NT, 6], F32, tag="lnbuf")
                if nl < GROUP:
                    nc.vector.memset(bnh, 0.0)
                    nc.vector.memset(lnbuf, 0.0)

                # paired DMAs: xt + w2, 2 nodes per descriptor
                xts, w2s = [], []
                for i0 in range(0, nl, 2):
                    np_pair = min(2, nl - i0)
                    xt2 = xt_pool.tile([128, 2, 514], BF16, tag="xt2")
                    w22 = w2_pool.tile([128, 2, KT, C_OUT], BF16, tag="w22")
                    n0 = nodes[i0]
                    if np_pair == 2:
                        nc.sync.dma_start(
                            out=xt2,
                            in_=xt_in[n0 : n0 + 2].rearrange("n p f -> p n f"))
                        nc.sync.dma_start(
                            out=w22,
                            in_=w2_c[n0 : n0 + 2].rearrange("n p k o -> p n k o"))
                    else:
                        nc.sync.dma_start(out=xt2[:, 0, :], in_=xt_in[n0])
                        nc.sync.dma_start(out=w22[:, 0, :, :], in_=w2_c[n0])
                    for q in range(np_pair):
                        xts.append(xt2[:, q, :])
                        w2s.append(w22[:, q, :, :])

                hsbs = []
                # GEMM, h copy (ACT), bn_stats (DVE)
                for i, n in enumerate(nodes):
                    xt_t = xts[i]
                    w2_t = w2s[i]
                    ph = ph_pool.tile([128, 2, T], F32, tag="ph")
                    for h in range(2):
                        for tap in range(KT):
                            nc.tensor.matmul(
                                ph[:, h, :],
                                lhsT=w2_t[:, tap, h * 128 : (h + 1) * 128],
                                rhs=xt_t[:, tap : tap + T],
                                start=(tap == 0),
                                stop=(tap == KT - 1),
                            )
                    h_sb = hsb_pool.tile([128, 2, T], BF16, tag="hsb")
                    hsbs.append(h_sb)
                    nc.scalar.activation(out=h_sb, in_=ph, func=AF.Copy)
                    for h in range(2):
                        nc.vector.bn_stats(out=bnh[:, i, h, :], in_=h_sb[:, h, :])
                return dict(nodes=nodes, bnh=bnh, lnbuf=lnbuf, xts=xts, hsbs=hsbs)

            def emit_phase2(state):
                nodes, bnh, lnbuf = state["nodes"], state["bnh"], state["lnbuf"]
                xts, hsbs = state["xts"], state["hsbs"]
                nl = len(nodes)

                # ---- GN finalize (from bn_stats even/odd moments) -----------
                # E[h]  = (me+mo)/2 ; E[h^2] = (cve+cvo)/512 + (me^2+mo^2)/2
                # St columns hold 2*E[h] and 2*E[h^2]; after the 32-partition
                # block-sum, scaling by 1/64 yields group E[h], E[h^2].
                # Elementwise chain on Pool; the two 32x32 block transposes on
                # DVE.
                me = bnh[:, :, :, 1]
                mo = bnh[:, :, :, 4]
                cve = bnh[:, :, :, 2]
                cvo = bnh[:, :, :, 5]
                St32 = fin_pool.tile([128, 32], F32, tag="St32")
                St = St32[:, 0 : GROUP * 4].rearrange("p (n h m) -> p n h m", h=2, m=2)
                nc.gpsimd.tensor_tensor(out=St[:, :, :, 0], in0=me, in1=mo, op=ALU.add)
                b_t = fin_pool.tile([128, GROUP, 2], F32, tag="bt")
                nc.gpsimd.tensor_tensor(out=b_t, in0=cve, in1=cvo, op=ALU.add)
                c_t = fin_pool.tile([128, GROUP, 2], F32, tag="ct")
                nc.gpsimd.tensor_tensor(out=c_t, in0=me, in1=me, op=ALU.mult)
                d_t = fin_pool.tile([128, GROUP, 2], F32, tag="dt")
                nc.gpsimd.tensor_tensor(out=d_t, in0=mo, in1=mo, op=ALU.mult)
                e_t = fin_pool.tile([128, GROUP, 2], F32, tag="et")
                nc.gpsimd.tensor_tensor(out=e_t, in0=c_t, in1=d_t, op=ALU.add)
                nc.gpsimd.scalar_tensor_tensor(
                    out=St[:, :, :, 1], in0=b_t, scalar=1.0 / 256.0, in1=e_t,
                    op0=ALU.mult, op1=ALU.add,
                )
                Tr = fin_pool.tile([128, 32], F32, tag="Tr")
                nc.vector.transpose(out=Tr, in_=St32)
                R = fin_pool.tile([128, 1], F32, tag="R")
                nc.vector.tensor_reduce(out=R, in_=Tr, axis=mybir.AxisListType.X, op=ALU.add)
                Rep = fin_pool.tile([128, 32], F32, tag="Rep")
                nc.gpsimd.tensor_scalar(
                    out=Rep, in0=ones32, scalar1=R[:, 0:1], scalar2=None, op0=ALU.mult
                )
                M0 = fin_pool.tile([128, 32], F32, tag="M0")
                nc.vector.transpose(out=M0, in_=Rep)
                Mn = fin_pool.tile([128, 32], F32, tag="Mn")
                nc.gpsimd.tensor_scalar(
                    out=Mn, in0=M0, scalar1=1.0 / 64.0, scalar2=None, op0=ALU.mult
                )
                MnV = Mn[:, 0 : GROUP * 4].rearrange("p (n s) -> p n s", s=4)
                mu = MnV[:, :, 0::2]      # [128, GROUP, 2]
                E2 = MnV[:, :, 1::2]
                musq = fin_pool.tile([128, GROUP, 2], F32, tag="musq")
                nc.gpsimd.tensor_tensor(out=musq, in0=mu, in1=mu, op=ALU.mult)
                veps = fin_pool.tile([128, GROUP, 2], F32, tag="veps")
                nc.gpsimd.scalar_tensor_tensor(
                    out=veps, in0=E2, scalar=EPS, in1=musq,
                    op0=ALU.add, op1=ALU.subtract,
                )
                rstd = _newton_rsqrt(nc, nc.gpsimd, fin_pool,
                                     veps.rearrange("p n s -> p (n s)"), magic, GROUP * 2)
                rstdV = rstd.rearrange("p (n s) -> p n s", s=2)
                gnsc = fin_pool.tile([128, GROUP * 2], F32, tag="gnsc")
                gnscV = gnsc.rearrange("p (n h) -> p n h", h=2)
                nc.gpsimd.tensor_tensor(out=gnscV, in0=rstdV, in1=gng_t, op=ALU.mult)
                gnbi = fin_pool.tile([128, GROUP * 2], F32, tag="gnbi")
                gnbiV = gnbi.rearrange("p (n h) -> p n h", h=2)
                nc.gpsimd.scalar_tensor_tensor(
                    out=gnbiV, in0=mu, scalar=-1.0, in1=gnscV, op0=ALU.mult, op1=ALU.mult
                )
                if gn_beta_nonzero:
                    nc.gpsimd.tensor_tensor(out=gnbiV, in0=gnbiV, in1=gnb_t, op=ALU.add)

                # ---- phase 2: GN apply (DVE 4x, in-place), y assembly (PE),
                #      y copy (Pool), LN stats (DVE) ------------------------
                ysbs = []
                for i, n in enumerate(nodes):
                    xt_t, h_sb = xts[i], hsbs[i]
                    for h in range(2):
                        nc.vector.tensor_scalar(
                            out=h_sb[:, h, :], in0=h_sb[:, h, :],
                            scalar1=gnsc[:, 2 * i + h : 2 * i + h + 1],
                            scalar2=gnbi[:, 2 * i + h : 2 * i + h + 1],
                            op0=ALU.mult, op1=ALU.add,
                        )
                    py = py_pool.tile([128, NT, C_OUT], F32, tag="py")
                    for j in range(NT):
                        nc.tensor.matmul(
                            py[:, j, :],
                            lhsT=xt_t[:, 1 + 128 * j : 1 + 128 * (j + 1)],
                            rhs=wres_t,
                            start=(j % 2 == 0), stop=False, skip_group_check=True,
                        )
                    for j in range(NT):
                        for h in range(2):
                            # transpose-by-identity as a regular bf16 matmul
                            # (1 cyc/row; transpose mode would force f32)
                            nc.tensor.matmul(
                                py[:, j, h * 128 : (h + 1) * 128],
                                lhsT=h_sb[:, h, 128 * j : 128 * (j + 1)],
                                rhs=ident_t,
                                start=False, stop=(h == 1), skip_group_check=True,
                            )
                    y_sb = y_pool.tile([128, NT, C_OUT], BF16, tag="ysb")
                    ysbs.append(y_sb)
                    # PSUM drain must be ACT or DVE (GPSIMD cannot touch PSUM)
                    nc.scalar.activation(out=y_sb, in_=py, func=AF.Copy)
                    # walrus requires bn_stats output = exactly 6 els/part
                    for j in range(NT):
                        nc.vector.bn_stats(out=lnbuf[:, i, j, :], in_=y_sb[:, j, :])

                # ---- LN finalize (Pool) -------------------------------------
                # per (node, j): mu = (me+mo)/2 ; E2 = (cve+cvo)/256+(me^2+mo^2)/2
                lme = lnbuf[:, :, :, 1]
                lmo = lnbuf[:, :, :, 4]
                lcve = lnbuf[:, :, :, 2]
                lcvo = lnbuf[:, :, :, 5]
                a_l = fin_pool.tile([128, GROUP, NT], F32, tag="al")
                nc.gpsimd.tensor_tensor(out=a_l, in0=lme, in1=lmo, op=ALU.add)
                mu_l = fin_pool.tile([128, GROUP, NT], F32, tag="mul")
                nc.gpsimd.tensor_scalar(
                    out=mu_l, in0=a_l, scalar1=0.5, scalar2=None, op0=ALU.mult
                )
                b_l = fin_pool.tile([128, GROUP, NT], F32, tag="bl")
                nc.gpsimd.tensor_tensor(out=b_l, in0=lcve, in1=lcvo, op=ALU.add)
                c_l = fin_pool.tile([128, GROUP, NT], F32, tag="cl")
                nc.gpsimd.tensor_tensor(out=c_l, in0=lme, in1=lme, op=ALU.mult)
                d_l = fin_pool.tile([128, GROUP, NT], F32, tag="dl")
                nc.gpsimd.tensor_tensor(out=d_l, in0=lmo, in1=lmo, op=ALU.mult)
                e_l = fin_pool.tile([128, GROUP, NT], F32, tag="el")
                nc.gpsimd.tensor_tensor(out=e_l, in0=c_l, in1=d_l, op=ALU.add)
                g_l = fin_pool.tile([128, GROUP, NT], F32, tag="gl")
                nc.gpsimd.scalar_tensor_tensor(
                    out=g_l, in0=b_l, scalar=1.0 / 128.0, in1=e_l,
                    op0=ALU.mult, op1=ALU.add,
                )   # = 2*E[y^2]
                musq_l = fin_pool.tile([128, GROUP, NT], F32, tag="musql")
                nc.gpsimd.tensor_tensor(out=musq_l, in0=mu_l, in1=mu_l, op=ALU.mult)
                musq2_l = fin_pool.tile([128, GROUP, NT], F32, tag="musq2l")
                nc.gpsimd.tensor_scalar(
                    out=musq2_l, in0=musq_l, scalar1=1.0, scalar2=EPS,
                    op0=ALU.mult, op1=ALU.subtract,
                )   # = mu^2 - EPS
                veps_l = fin_pool.tile([128, GROUP, NT], F32, tag="vepsl")
                nc.gpsimd.scalar_tensor_tensor(
                    out=veps_l, in0=g_l, scalar=0.5, in1=musq2_l,
                    op0=ALU.mult, op1=ALU.subtract,
                )   # = E[y^2] - mu^2 + EPS
                rsig = _newton_rsqrt(
                    nc, nc.vector, fin_pool,
                    veps_l.rearrange("p n j -> p (n j)"), magic, GROUP * NT
                )
                rsigV = rsig.rearrange("p (n j) -> p n j", j=NT)
                nb = fin_pool.tile([128, GROUP * NT], F32, tag="nb")
                nbV = nb.rearrange("p (n j) -> p n j", j=NT)
                nc.gpsimd.scalar_tensor_tensor(
                    out=nbV, in0=mu_l, scalar=-1.0, in1=rsigV, op0=ALU.mult, op1=ALU.mult
                )
                return dict(nodes=nodes, ysbs=ysbs, rsig=rsig, nb=nb)

            def emit_phase3(state):
                nodes, ysbs = state["nodes"], state["ysbs"]
                rsig, nb = state["rsig"], state["nb"]
                nl = len(nodes)
                # ---- phase 3: LN apply + gelu + paired store ---------------
                osb2 = None
                for i, n in enumerate(nodes):
                    y_sb = ysbs[i]
                    if i % 2 == 0:
                        osb2 = out_pool.tile([128, 2, NT, C_OUT], BF16, tag="osb2")
                    o_sb = osb2[:, i % 2, :, :]
                    if not ln_nontrivial:
                        u = scr_pool.tile([128, NT, C_OUT], BF16, tag="u")
                        for j in range(NT):
                            nc.gpsimd.tensor_scalar(
                                out=u[:, j, :], in0=y_sb[:, j, :],
                                scalar1=rsig[:, NT * i + j : NT * i + j + 1],
                                scalar2=nb[:, NT * i + j : NT * i + j + 1],
                                op0=ALU.mult, op1=ALU.add,
                            )
                        nc.scalar.activation(out=o_sb, in_=u, func=AF.Gelu)
                    else:
                        u = scr_pool.tile([128, NT, C_OUT], F32, tag="u")
                        for j in range(NT):
                            nc.vector.tensor_scalar(
                                out=u[:, j, :], in0=y_sb[:, j, :],
                                scalar1=rsig[:, NT * i + j : NT * i + j + 1],
                                scalar2=nb[:, NT * i + j : NT * i + j + 1],
                                op0=ALU.mult, op1=ALU.add,
                            )
                            nc.vector.tensor_tensor(
                                out=u[:, j, :], in0=u[:, j, :], in1=lng_t, op=ALU.mult
                            )
                            nc.vector.tensor_tensor(
                                out=u[:, j, :], in0=u[:, j, :], in1=lnb_t, op=ALU.add
                            )
                            nc.scalar.activation(
                                out=o_sb[:, j, :], in_=u[:, j, :], func=AF.Gelu
                            )
                    if i % 2 == 1:
                        n0 = nodes[i - 1]
                        nc.sync.dma_start(
                            out=out_d[n0 : n0 + 2].rearrange("n p j o -> p n j o"),
                            in_=osb2)
                    elif i == nl - 1:
                        nc.sync.dma_start(out=out_d[n], in_=osb2[:, 0, :, :])

            # Software pipeline, 3 stages with 2-group lookahead:
            #   ph1(g+1) || fin+ph2(g) || ph3(g-1)
            # so gelu/store never wait on the LN finalize just computed.
            p1 = p2 = None
            for rep in range(repeat):
                for g in range(n_groups):
                    cur = emit_phase1(g)
                    if p1 is not None:
                        st2 = emit_phase2(p1)
                        if p2 is not None:
                            emit_phase3(p2)
                        p2 = st2
                    p1 = cur
            st2 = emit_phase2(p1)
            if p2 is not None:
                emit_phase3(p2)
            emit_phase3(st2)

    _fix_multiwaits(nc)
    return nc


_CACHE: dict = {}


def _fold_consts(x, A, adj_residual, dw_weights, W_pw, W_conv, gn_gamma, gn_beta,
                 ln_gamma, ln_beta, W_res):
    """Host-side parameter folding. Returns (consts, flags, xt_host)."""
    x = np.asarray(x, np.float32)
    A = np.asarray(A, np.float32)
    adj_residual = np.asarray(adj_residual, np.float32)
    dw_weights = np.asarray(dw_weights, np.float32)
    W_pw = np.asarray(W_pw, np.float32)
    W_conv = np.asarray(W_conv, np.float32)
    gn_gamma = np.asarray(gn_gamma, np.float32)
    gn_beta = np.asarray(gn_beta, np.float32)
    ln_gamma = np.asarray(ln_gamma, np.float32)
    ln_beta = np.asarray(ln_beta, np.float32)
    W_res = np.asarray(W_res, np.float32)

    A_eff = A + np.tanh(adj_residual) * 0.3
    A_eff = A_eff / np.clip(np.abs(A_eff).sum(-1, keepdims=True), 1.0, None)
    rowsum = A_eff.sum(-1)                                   # [K, N]
    W_pw_r = W_pw.reshape(C_OUT, K, C_IN)                    # [o, k, c]
    W_effT = np.einsum("kn,kc,okc->nco", rowsum, dw_weights, W_pw_r)
    WC = W_conv[:, 0, :]                                     # [o, tap]
    W2 = W_effT[:, :, None, :] * WC.T[None, None, :, :]      # [n, c, tap, o]
    w2_host = np.ascontiguousarray(W2).astype(ml_dtypes.bfloat16)

    xt = np.zeros((B, N, 128, 514), np.float32)
    xt[:, :, :, 1:513] = np.transpose(x, (0, 2, 3, 1))       # [b, n, c, t]
    xt_host = xt.astype(ml_dtypes.bfloat16)

    wres_host = np.ascontiguousarray(W_res.T).astype(ml_dtypes.bfloat16)
    ident = np.eye(128, dtype=np.float32).astype(ml_dtypes.bfloat16)

    p = np.arange(128)
    gng = np.zeros((128, GROUP, 2), np.float32)
    gnb = np.zeros((128, GROUP, 2), np.float32)
    for h in range(2):
        gng[:, :, h] = gn_gamma[h * 128 + p][:, None]
        gnb[:, :, h] = gn_beta[h * 128 + p][:, None]
    gng = gng.reshape(128, GROUP * 2)
    gnb = gnb.reshape(128, GROUP * 2)

    ln_nontrivial = not (np.all(ln_gamma == 1.0) and np.all(ln_beta == 0.0))
    gn_beta_nonzero = bool(np.any(gn_beta != 0.0))
    lng = np.broadcast_to(ln_gamma[None, :], (128, C_OUT)).astype(np.float32).copy()
    lnb = np.broadcast_to(ln_beta[None, :], (128, C_OUT)).astype(np.float32).copy()

    consts = {
        "w2": w2_host, "wres": wres_host, "ident": ident,
        "gng": gng, "gnb": gnb, "lng": lng, "lnb": lnb,
    }
    return consts, (gn_beta_nonzero, ln_nontrivial), xt_host


def prepare(x, A, adj_residual, dw_weights, W_pw, W_conv, gn_gamma, gn_beta,
            ln_gamma, ln_beta, W_res, repeat: int = 1):
    """Host-side parameter folding + input staging. Returns (nc, in_maps)."""
    consts, (gn_beta_nonzero, ln_nontrivial), xt_host = _fold_consts(
        x, A, adj_residual, dw_weights, W_pw, W_conv, gn_gamma, gn_beta,
        ln_gamma, ln_beta, W_res)
    hsh = hashlib.sha1()
    for k in sorted(consts):
        hsh.update(np.ascontiguousarray(consts[k]).tobytes())
    key = (gn_beta_nonzero, ln_nontrivial, repeat, hsh.hexdigest())
    if key not in _CACHE:
        _CACHE[key] = _build_nc(consts, gn_beta_nonzero, ln_nontrivial,
                                repeat=repeat)
    nc = _CACHE[key]

    in_maps = [{"xt": xt_host[b]} for b in range(B)]
    return nc, in_maps


def _warm_devices():
    """A previously crashed process can leave the remote NeuronCores in an
    unrecoverable state; run a sacrificial tiny op (with retries) so the real
    launch lands on healthy devices."""
    import time as _time
    import jax
    import jax.numpy as jnp
    for attempt in range(4):
        try:
            for d in jax.devices()[:N_CORES]:
                y = jax.device_put(jnp.ones((2,)), d) + 1.0
                np.asarray(y)
            return
        except Exception:
            _time.sleep(4.0)


def kernel(x, A, adj_residual, dw_weights, W_pw, W_conv, gn_gamma, gn_beta,
           ln_gamma, ln_beta, W_res):
    nc, in_maps = prepare(x, A, adj_residual, dw_weights, W_pw, W_conv,
                          gn_gamma, gn_beta, ln_gamma, ln_beta, W_res)
    _warm_devices()
    import time as _time
    last_err = None
    for attempt in range(2):
        try:
            res = run_bass_kernel_spmd(nc, in_maps, core_ids=list(range(N_CORES)))
            break
        except Exception as e:
            last_err = e
            _time.sleep(4.0)
            _warm_devices()
    else:
        raise last_err
    # out per core: [N, p, j, o] bf16; t = j*128 + p
    outs = []
    for b in range(B):
        arr = np.asarray(res.results[b]["out"])      # [N, 128, NT, C_OUT]
        arr = arr.transpose(2, 1, 0, 3).reshape(T, N, C_OUT)
        outs.append(arr)
    return np.stack(outs, axis=0).astype(np.float32)
